# revision 7
# baseline (speedup 1.0000x reference)
"""Bass TRN2 kernel for nn_LinearColumnwise: out = concat_rows(input) @ weight + bias.

Sharding: input [8, 2048, 4096] is row-sharded -- core i computes
out[i*2048:(i+1)*2048, :] = input[i] @ weight + bias locally; no collectives.

Per-core kernel: bf16 GEMM (host-cast), fp32 PSUM accumulation, bias folded in
as an extra contraction k-tile (ones row in xT, bias row in w). Raw bass with
hand-placed semaphores: every instruction carries at most one wait or one
update (this toolchain rejects >2 sync commands per instruction).
"""

import numpy as np
import ml_dtypes

P = 128
M = 2048          # rows per core
K = 4096          # contraction
N = 4096          # out features
KT = K // P + 1   # 33: one extra k-tile carries the bias trick
KA = KT * P       # 4224 padded contraction
NT = 512          # psum-bank n tile
NNT = N // NT     # 8 n tiles
MS = M // P       # 16 m subtiles
N_CORES = 8
XT_CHUNKS = 8     # xT load split for queue parallelism

_cached = None


def _build():
    import concourse.bass as bass
    import concourse.mybir as mybir

    f32 = mybir.dt.float32
    bf16 = mybir.dt.bfloat16

    nc = bass.Bass()
    xt_d = nc.declare_dram_parameter("xt", [KA, M], bf16, isOutput=False)
    w_d = nc.declare_dram_parameter("w", [KA, N], bf16, isOutput=False)
    out_d = nc.declare_dram_parameter("out", [M, N], f32, isOutput=True)

    xt_sb = nc.alloc_sbuf_tensor("xt_sb", [P, KT, M], bf16).ap()
    w_sb = [nc.alloc_sbuf_tensor(f"w_sb{b}", [P, KT, NT], bf16).ap() for b in range(2)]
    ps = [nc.alloc_psum_tensor(f"ps{i}", [P, NT], f32).ap() for i in range(8)]
    stage = [nc.alloc_sbuf_tensor(f"stage{i}", [P, NT], f32).ap() for i in range(4)]

    xt_r = xt_d.rearrange("(kt p) m -> p kt m", p=P)

    # kt chunk boundaries for xT loads
    bounds = [round(i * KT / XT_CHUNKS) for i in range(XT_CHUNKS + 1)]

    with nc.Block() as block:
        xc_sems = [nc.semaphore(f"xc{c}").__enter__() for c in range(XT_CHUNKS)]
        w_sems = [nc.semaphore(f"wsem{b}").__enter__() for b in range(2)]
        pe_sem = nc.semaphore("pe_grp").__enter__()
        cp_sem = nc.semaphore("copied").__enter__()
        ev_sem = nc.semaphore("evict").__enter__()

        @block.sync
        def _(sp):
            # first weight slab before the bulk xT load so PE can start asap
            sp.dma_start(
                out=w_sb[0][:],
                in_=w_d[:, 0:NT].rearrange("(kt p) n -> p kt n", p=P),
            ).then_inc(w_sems[0], 16)
            for c in range(XT_CHUNKS):
                lo, hi = bounds[c], bounds[c + 1]
                sp.dma_start(
                    out=xt_sb[:, lo:hi, :], in_=xt_r[:, lo:hi, :]
                ).then_inc(xc_sems[c], 16)
            for nt in range(1, NNT):
                if nt >= 2:
                    # slab buffer nt%2 reused: PE must be done with slab nt-2
                    sp.wait_ge(pe_sem, 4 * (nt - 1))
                sp.dma_start(
                    out=w_sb[nt % 2][:],
                    in_=w_d[:, nt * NT : (nt + 1) * NT].rearrange(
                        "(kt p) n -> p kt n", p=P
                    ),
                ).then_inc(w_sems[nt % 2], 16)

        @block.tensor
        def _(te):
            seen_chunk = [False] * XT_CHUNKS
            g = 0
            for nt in range(NNT):
                te.wait_ge(w_sems[nt % 2], 16 * (nt // 2 + 1))
                for mq in range(4):
                    if g >= 2:
                        # bank set g%2 reused from group g-2: DVE copied it out
                        te.wait_ge(cp_sem, 4 * (g - 1))
                    bank0 = (g % 2) * 4
                    inst = None
                    for kt in range(KT):
                        c = next(
                            i for i in range(XT_CHUNKS) if bounds[i] <= kt < bounds[i + 1]
                        )
                        if not seen_chunk[c]:
                            te.wait_ge(xc_sems[c], 16)
                            seen_chunk[c] = True
                        for ms in range(4):
                            m0 = (mq * 4 + ms) * P
                            inst = te.matmul(
                                ps[bank0 + ms][:],
                                xt_sb[:, kt, m0 : m0 + P],
                                w_sb[nt % 2][:, kt, :],
                                start=(kt == 0),
                                stop=(kt == KT - 1),
                            )
                    inst.then_inc(pe_sem, 1)
                    g += 1

        @block.vector
        def _(ve):
            for g in range(4 * NNT):
                ve.wait_ge(pe_sem, g + 1)
                if g >= 1:
                    # staging slots reused every group: out-DMAs of g-1 done
                    ve.wait_ge(ev_sem, 64 * g)
                b = (g % 2) * 4
                for ms in range(4):
                    ve.tensor_copy(stage[ms][:], ps[b + ms][:]).then_inc(cp_sem, 1)

        @block.scalar
        def _(act):
            for g in range(4 * NNT):
                nt, mq = divmod(g, 4)
                act.wait_ge(cp_sem, 4 * (g + 1))
                for ms in range(4):
                    m0 = (mq * 4 + ms) * P
                    act.dma_start(
                        out=out_d[m0 : m0 + P, nt * NT : (nt + 1) * NT],
                        in_=stage[ms][:],
                    ).then_inc(ev_sem, 16)
            act.wait_ge(ev_sem, 16 * 4 * 4 * NNT)

    return nc


def _get_nc():
    global _cached
    if _cached is None:
        _cached = _build()
    return _cached


def _prep_core_input(x_core, w_aug):
    # [2048, 4096] f32 -> transposed, bf16, padded with ones row-tile
    xt = np.zeros((KA, M), dtype=ml_dtypes.bfloat16)
    xt[:K] = np.ascontiguousarray(x_core.T).astype(ml_dtypes.bfloat16)
    xt[K] = np.float32(1.0)
    return {"xt": xt, "w": w_aug}


def kernel(input, weight, bias):
    from concourse.bass_utils import run_bass_kernel_spmd

    assert input.shape == (N_CORES, M, K)
    nc = _get_nc()

    w_aug = np.zeros((KA, N), dtype=ml_dtypes.bfloat16)
    w_aug[:K] = weight.astype(ml_dtypes.bfloat16)
    w_aug[K] = bias.astype(ml_dtypes.bfloat16)

    in_maps = [_prep_core_input(input[i], w_aug) for i in range(N_CORES)]
    res = run_bass_kernel_spmd(nc, in_maps, list(range(N_CORES)))
    return np.concatenate([res.results[i]["out"] for i in range(N_CORES)], axis=0)
